# revision 3
# baseline (speedup 1.0000x reference)
"""DRRN scoring network on 8 Trainium2 NeuronCores (data-parallel).

Per core: 32 states x 3 state-encoders + 256 actions. The three state
GRUs and the action GRU (8 sequential 32-step chunks) run concurrently
in the 128x128 PE array via column tiling (32 batch rows per encoder).
Embedding rows are fetched with dma_gather in bf16-transpose mode so the
gi matmul consumes them directly as the stationary operand. Hidden-state
recurrence keeps h in two layouts: [batch, H] for gate math and [H,
batch] (PE-transposed each step) for the next step's matmul.
"""
import numpy as np
import ml_dtypes
import concourse.bacc as bacc
import concourse.mybir as mybir
from concourse.tile import TileContext
from concourse import masks as bass_masks
from concourse.bass_utils import run_bass_kernel_spmd

dt = mybir.dt
F32, BF16, I16, F32R = dt.float32, dt.bfloat16, dt.int16, dt.float32r
bf = ml_dtypes.bfloat16

V, E, H = 32000, 128, 256
B, S = 256, 256
A, SA = 8, 32
NCORES = 8
BL = B // NCORES            # states per core
AR = BL * A                 # act rows per core
NSTEP = S
NIDX = NSTEP * 128
NSLAB = 8
SLAB = NIDX // NSLAB
H3 = 3 * H
H2 = 2 * H

Sig = mybir.ActivationFunctionType.Sigmoid
Tanh = mybir.ActivationFunctionType.Tanh
Relu = mybir.ActivationFunctionType.Relu
Ident = mybir.ActivationFunctionType.Identity
MUL = mybir.AluOpType.mult
ADD = mybir.AluOpType.add
SUB = mybir.AluOpType.subtract


def build_nc(nreps=1):
    nc = bacc.Bacc("TRN2", target_bir_lowering=False, debug=False)

    d_emb = nc.declare_dram_parameter("embb", [V, E], BF16, isOutput=False)
    d_idx = nc.declare_dram_parameter("idx", [128, NIDX // 16], I16, isOutput=False)
    d_wih = nc.declare_dram_parameter("wihT", [E, 4, H3], BF16, isOutput=False)
    d_whh = nc.declare_dram_parameter("whhT", [128, 4, 2, H3], BF16, isOutput=False)
    d_sel = nc.declare_dram_parameter("sel", [4, 128], F32R, isOutput=False)
    d_brz = nc.declare_dram_parameter("brz", [4, H2], F32R, isOutput=False)
    d_bin = nc.declare_dram_parameter("bin", [4, H], F32R, isOutput=False)
    d_bhn = nc.declare_dram_parameter("bhn", [4, H], F32R, isOutput=False)
    d_m = nc.declare_dram_parameter("mask", [128, NSTEP], F32, isOutput=False)
    d_mn = nc.declare_dram_parameter("maskn", [128, NSTEP], F32, isOutput=False)
    d_hw = nc.declare_dram_parameter("hWT", [128, 2, 8, 128], BF16, isOutput=False)
    d_hb = nc.declare_dram_parameter("hb", [128, 2], F32, isOutput=False)
    d_sw = nc.declare_dram_parameter("sWT", [128, 2], F32R, isOutput=False)
    d_sb = nc.declare_dram_parameter("sbias", [1, 1], F32, isOutput=False)
    d_q = nc.declare_dram_parameter("q", [1, AR], F32, isOutput=True)

    with TileContext(nc) as tc:
        with tc.tile_pool(name="w", bufs=1) as wp, \
             tc.tile_pool(name="xp", bufs=1) as xp, \
             tc.tile_pool(name="st", bufs=1) as stp, \
             tc.tile_pool(name="rot", bufs=2) as rp, \
             tc.tile_pool(name="ps", bufs=2, space="PSUM") as ps:

            t_idx = wp.tile([128, NIDX // 16], I16)
            nc.sync.dma_start(out=t_idx[:], in_=d_idx[:])
            t_wih = wp.tile([E, 4, H3], BF16)
            nc.sync.dma_start(out=t_wih[:], in_=d_wih[:])
            t_whh = wp.tile([128, 4, 2, H3], BF16)
            nc.sync.dma_start(out=t_whh[:], in_=d_whh[:])
            t_sel = wp.tile([4, 128], F32R)
            nc.sync.dma_start(out=t_sel[:], in_=d_sel[:])
            t_brz = wp.tile([4, H2], F32R)
            nc.sync.dma_start(out=t_brz[:], in_=d_brz[:])
            t_bin = wp.tile([4, H], F32R)
            nc.sync.dma_start(out=t_bin[:], in_=d_bin[:])
            t_bhn = wp.tile([4, H], F32R)
            nc.sync.dma_start(out=t_bhn[:], in_=d_bhn[:])
            t_m = wp.tile([128, NSTEP], F32)
            nc.sync.dma_start(out=t_m[:], in_=d_m[:])
            t_mn = wp.tile([128, NSTEP], F32)
            nc.sync.dma_start(out=t_mn[:], in_=d_mn[:])
            t_hw = wp.tile([128, 2, 8, 128], BF16)
            nc.sync.dma_start(out=t_hw[:], in_=d_hw[:])
            t_hb = wp.tile([128, 2], F32)
            nc.sync.dma_start(out=t_hb[:], in_=d_hb[:])
            t_sw = wp.tile([128, 2], F32R)
            nc.sync.dma_start(out=t_sw[:], in_=d_sw[:])
            t_sb = wp.tile([1, 1], F32)
            nc.sync.dma_start(out=t_sb[:], in_=d_sb[:])
            t_id = wp.tile([128, 128], BF16)
            bass_masks.make_identity(nc, t_id[:])

            xts = []
            for s in range(NSLAB):
                xt = xp.tile([128, 1, SLAB], BF16, tag=f"xt{s}")
                nc.gpsimd.dma_gather(
                    out_ap=xt[:], in_ap=d_emb[:],
                    idxs_ap=t_idx[:, (SLAB // 16) * s:(SLAB // 16) * (s + 1)],
                    num_idxs=SLAB, num_idxs_reg=SLAB, elem_size=E,
                    transpose=True,
                )
                xts.append(xt)

            t_aT = [stp.tile([128, AR], BF16, tag=f"aT{k}") for k in range(2)]

            for rep in range(nreps):
                h_A = rp.tile([128, H], BF16, tag="hA")
                nc.vector.memset(h_A[:], 0.0)
                h_T = [rp.tile([128, 128], BF16, tag=f"hT{k}") for k in range(2)]
                for k in range(2):
                    nc.vector.memset(h_T[k][:], 0.0)

                for t in range(NSTEP):
                    xslab = xts[t // (NSTEP // NSLAB)]
                    xoff = (t % (NSTEP // NSLAB)) * 128
                    p_rz = ps.tile([128, H2], F32, tag="prz")
                    p_gin = ps.tile([128, H], F32, tag="pgin")
                    p_ghn = ps.tile([128, H], F32, tag="pghn")

                    nc.tensor.matmul(p_rz[:], t_sel[:], t_brz[:], start=True, stop=False)
                    nc.tensor.matmul(p_gin[:], t_sel[:], t_bin[:], start=True, stop=False)
                    nc.tensor.matmul(p_ghn[:], t_sel[:], t_bhn[:], start=True, stop=False)

                    for e in range(4):
                        xs = xslab[:, 0, xoff + 32 * e:xoff + 32 * (e + 1)]
                        nc.tensor.matmul(p_rz[32 * e:32 * (e + 1), :], xs,
                                         t_wih[:, e, 0:H2], start=False, stop=False,
                                         tile_position=(0, 32 * e))
                        nc.tensor.matmul(p_gin[32 * e:32 * (e + 1), :], xs,
                                         t_wih[:, e, H2:H3], start=False, stop=True,
                                         tile_position=(0, 32 * e))
                    for k in range(2):
                        last = (k == 1)
                        for e in range(4):
                            hs = h_T[k][:, 32 * e:32 * (e + 1)]
                            nc.tensor.matmul(p_rz[32 * e:32 * (e + 1), :], hs,
                                             t_whh[:, e, k, 0:H2], start=False,
                                             stop=last, tile_position=(0, 32 * e))
                            nc.tensor.matmul(p_ghn[32 * e:32 * (e + 1), :], hs,
                                             t_whh[:, e, k, H2:H3], start=False,
                                             stop=last, tile_position=(0, 32 * e))

                    s_rz = rp.tile([128, H2], BF16, tag="srz")
                    nc.scalar.activation(s_rz[:], p_rz[:], Sig)
                    t1 = rp.tile([128, H], BF16, tag="t1")
                    nc.vector.tensor_tensor(t1[:], s_rz[:, 0:H], p_ghn[:], op=MUL)
                    t2 = rp.tile([128, H], BF16, tag="t2")
                    nc.vector.tensor_tensor(t2[:], p_gin[:], t1[:], op=ADD)
                    s_n = rp.tile([128, H], BF16, tag="sn")
                    nc.scalar.activation(s_n[:], t2[:], Tanh)
                    # z'' = (1-z)*m == z*(-m) + m ; frozen rows get exactly 0
                    s_zp = rp.tile([128, H], BF16, tag="szp")
                    nc.vector.tensor_scalar(s_zp[:], s_rz[:, H:H2],
                                            t_mn[:, t:t + 1], t_m[:, t:t + 1],
                                            MUL, ADD)
                    s_d = rp.tile([128, H], BF16, tag="sd")
                    nc.vector.tensor_tensor(s_d[:], s_n[:], h_A[:], op=SUB)
                    s_u = rp.tile([128, H], BF16, tag="su")
                    nc.vector.tensor_tensor(s_u[:], s_zp[:], s_d[:], op=MUL)
                    h_A2 = rp.tile([128, H], BF16, tag="hA")
                    nc.vector.tensor_tensor(h_A2[:], h_A[:], s_u[:], op=ADD)
                    h_A = h_A2

                    p_tr = ps.tile([128, 256], BF16, tag="ptr")
                    h_T = [rp.tile([128, 128], BF16, tag=f"hT{k}") for k in range(2)]
                    for k in range(2):
                        nc.tensor.transpose(p_tr[:, 128 * k:128 * (k + 1)],
                                            h_A[:, 128 * k:128 * (k + 1)], t_id[:])
                        nc.scalar.copy(h_T[k][:], p_tr[:, 128 * k:128 * (k + 1)])

                    if t % SA == SA - 1:
                        g = t // SA
                        for k in range(2):
                            nc.vector.tensor_copy(t_aT[k][:, 32 * g:32 * (g + 1)],
                                                  h_T[k][:, 96:128])
                        if t != NSTEP - 1:
                            for k in range(2):
                                nc.vector.memset(h_T[k][:, 96:128], 0.0)
                            nc.vector.memset(h_A[96:128, :], 0.0)

                # final MLP: z = relu(W_h @ concat(state_rep, act) + b), q = W_s @ z + b_s
                srep = []
                for kk in range(6):
                    enc, kch = kk // 2, kk % 2
                    sr = stp.tile([128, AR], BF16, tag=f"srep{kk}")
                    for a in range(A):
                        nc.vector.tensor_copy(sr[:, a::A],
                                              h_T[kch][:, 32 * enc:32 * enc + BL])
                    srep.append(sr)
                p_z = [ps.tile([128, AR], F32, tag=tg) for tg in ("prz", "pgin")]
                for m in range(2):
                    for kk in range(8):
                        rhs = srep[kk][:] if kk < 6 else t_aT[kk - 6][:]
                        nc.tensor.matmul(p_z[m][:], t_hw[:, m, kk, :], rhs,
                                         start=(kk == 0), stop=(kk == 7))
                z_s = [stp.tile([128, AR], F32R, tag=f"zs{m}") for m in range(2)]
                for m in range(2):
                    nc.scalar.activation(z_s[m][:], p_z[m][:], Relu,
                                         bias=t_hb[:, m:m + 1])
                p_q = ps.tile([1, AR], F32, tag="pghn")
                for m in range(2):
                    nc.tensor.matmul(p_q[:], t_sw[:, m:m + 1], z_s[m][:],
                                     start=(m == 0), stop=(m == 1))
                q_s = stp.tile([1, AR], F32, tag="qs")
                nc.scalar.activation(q_s[:], p_q[:], Ident, bias=t_sb[:])
                nc.sync.dma_start(out=d_q[:], in_=q_s[:])

    nc.compile()
    return nc


def _wrap_idx(tokens_flat):
    """tokens_flat: [NIDX] int -> wrapped [128, NIDX//16] int16 (16-part
    blocks per 4096-token slab, replicated across the 8 gpsimd cores)."""
    out = np.zeros((128, NIDX // 16), np.int16)
    for s in range(NSLAB):
        blk = tokens_flat[SLAB * s:SLAB * (s + 1)].reshape(SLAB // 16, 16).T
        out[:, (SLAB // 16) * s:(SLAB // 16) * (s + 1)] = np.tile(blk, (8, 1))
    return out


def prep_inputs(obs_tokens, obs_len, look_tokens, look_len, inv_tokens, inv_len,
                act_tokens, act_len, emb, Wih, Whh, bih, bhh,
                hidden_W, hidden_b, scorer_W, scorer_b):
    npf = np.asarray
    obs_tokens = npf(obs_tokens); look_tokens = npf(look_tokens)
    inv_tokens = npf(inv_tokens); act_tokens = npf(act_tokens)
    obs_len = npf(obs_len); look_len = npf(look_len)
    inv_len = npf(inv_len); act_len = npf(act_len)
    emb = npf(emb, np.float32)
    Wih = npf(Wih, np.float32); Whh = npf(Whh, np.float32)
    bih = npf(bih, np.float32); bhh = npf(bhh, np.float32)
    hidden_W = npf(hidden_W, np.float32); hidden_b = npf(hidden_b, np.float32)
    scorer_W = npf(scorer_W, np.float32); scorer_b = npf(scorer_b, np.float32)

    emb_bf = emb.astype(bf)
    wihT = np.ascontiguousarray(Wih.transpose(2, 0, 1)).astype(bf)      # [E,4,3H]
    whhT = np.ascontiguousarray(
        Whh.transpose(2, 0, 1).reshape(2, 128, 4, H3).transpose(1, 2, 0, 3)
    ).astype(bf)                                                        # [128,4,2,3H]
    sel = np.zeros((4, 128), np.float32)
    for e in range(4):
        sel[e, 32 * e:32 * (e + 1)] = 1.0
    brz = (bih[:, 0:H2] + bhh[:, 0:H2]).astype(np.float32)
    bin_ = bih[:, H2:H3].copy()
    bhn = bhh[:, H2:H3].copy()
    hWT = np.ascontiguousarray(
        hidden_W.T.reshape(8, 128, 2, 128).transpose(1, 2, 0, 3)
    ).astype(bf)                                                        # [128,2,8,128]
    hb = hidden_b.reshape(2, 128).T.copy()                              # [128,2]
    sWT = scorer_W.reshape(2, 128).T.copy()                             # [128,2]
    sbias = scorer_b.reshape(1, 1)

    enc_tok = [obs_tokens, look_tokens, inv_tokens]
    enc_len = [obs_len, look_len, inv_len]

    in_maps = []
    for c in range(NCORES):
        sl = slice(BL * c, BL * (c + 1))
        asl = slice(AR * c, AR * (c + 1))
        at = act_tokens[asl]          # [256, 32]
        al = act_len[asl]             # [256]
        toks = np.zeros((NSTEP, 128), np.int64)
        for e in range(3):
            # [BL, S] -> per step t the column t
            toks[:, 32 * e:32 * (e + 1)] = enc_tok[e][sl].T
        # act: step t -> chunk g=t//SA rows [32g..32g+32), col t%SA
        atr = at.reshape(A, 32, SA)   # [chunk, row, step]
        toks[:, 96:128] = atr.transpose(0, 2, 1).reshape(NSTEP, 32)
        m = np.zeros((128, NSTEP), np.float32)
        tt = np.arange(NSTEP)
        for e in range(3):
            m[32 * e:32 * (e + 1), :] = (tt[None, :] < enc_len[e][sl][:, None])
        alr = al.reshape(A, 32)       # [chunk, row]
        for g in range(A):
            m[96:128, SA * g:SA * (g + 1)] = (
                np.arange(SA)[None, :] < alr[g][:, None])
        in_maps.append({
            "embb": emb_bf,
            "idx": _wrap_idx(toks.reshape(-1)),
            "wihT": wihT, "whhT": whhT, "sel": sel,
            "brz": brz, "bin": bin_, "bhn": bhn,
            "mask": m, "maskn": -m,
            "hWT": hWT, "hb": hb, "sWT": sWT, "sbias": sbias,
        })
    return in_maps


_NC_CACHE = {}


def kernel(**inputs):
    nreps = 1
    if nreps not in _NC_CACHE:
        _NC_CACHE[nreps] = build_nc(nreps)
    nc = _NC_CACHE[nreps]
    in_maps = prep_inputs(**inputs)
    res = run_bass_kernel_spmd(nc, in_maps, list(range(NCORES)))
    q = np.concatenate([np.asarray(res.results[c]["q"][0], np.float32)
                        for c in range(NCORES)])
    return q.reshape(B, A)
